# revision 6
# baseline (speedup 1.0000x reference)
"""Trainium2 Bass kernel for nn_CausalLayer (bilinear causal mixing layer).

Math (per batch b):
    E = ae[x]                                # [L, D] gather
    S[i,j] = E_i @ w @ E_j                   # bilinear pairwise score
    coef[i,j] = (i+1)/(j+1) for i<j else 0
    res[:,j] = bx[:,j] + sum_i coef[i,j]*S[i,j]*bx[:,i]

Chunked linear-attention decomposition (chunk C=128), with A = E @ w
precomputed on the host into the gather table (row v = [ae[v] | (ae@w)[v]]):
    S_c   = A_c @ E_c^T              [C, C]   local score block
    St_c  = S_c * (gi+1) * mask(i<j)
    M_c   = sum_{c'<c} ((gi+1)A_c')^T @ bx_c'  [D, H]  running state (PSUM)
    corr_c = (St_c^T @ bx_c + E_c @ M_c) / (gj+1)
    res   = bx + corr                          (the +bx happens on the HOST)

Per chunk the PE does: 2 transposes (E^T, A^T), the S block, and three
K<=128 x N=768 streams (St@bx, E@M, M-update) -- ~2240 cycles. The PE on
this part runs at a fixed ~1.2 GHz (HAM clock-gate never unthrottles;
verified with a dense warmup burst), so emission is software-pipelined:
chunk G+1's transposes/S interleave into chunk G's big streams so the
s_p -> St(DVE) -> St@bx chain never stalls the PE.

Output is stored bf16 (gate is 2e-2; bf16 adds ~4e-4) and upcast + bias-
added on the host. Scalar/vector/gpsimd work is split so every engine
stays under the PE window: scalar={E^T copy, fixup-lo, M_s-lo},
vector={A^T copy, St, fixup-hi, M_s-hi}, gpsimd={gathers, Ap scale}.

Sharding: batch-parallel, 2 of 16 batches per NeuronCore across 8 cores;
the gather table / w / masks are replicated. No cross-core communication.
"""

import os
import sys

for _p in ("/opt/trn_rl_repo", "/root/.axon_site/_ro/trn_rl_repo"):
    if os.path.isdir(_p) and _p not in sys.path:
        sys.path.insert(0, _p)

import numpy as np

B, L, H = 16, 2048, 768
V, D = 30000, 64
NCORES = 8
BPC = B // NCORES          # batches per core
C = 128                    # chunk (tile) size along sequence
NCH = L // C               # chunks per batch
ROWS = BPC * L             # bx rows per core
NCHT = BPC * NCH           # chunks per core

_compiled = {}


def _np_consts():
    i = np.arange(C, dtype=np.float64)
    consts = np.zeros((C, 2 * NCH), np.float32)
    for c in range(NCH):
        gi = c * C + i
        consts[:, c] = (gi + 1.0).astype(np.float32)
        consts[:, NCH + c] = (1.0 / (gi + 1.0)).astype(np.float32)
    base01 = (i[:, None] < i[None, :]).astype(np.float32)
    return base01, consts


def _build():
    """Build + compile the per-core Bass module (SPMD: same program, 8 cores)."""
    key = "v4"
    if key in _compiled:
        return _compiled[key]

    import concourse.bacc as bacc
    import concourse.bass as bass
    import concourse.mybir as mybir
    import concourse.tile as tile
    from concourse.masks import make_identity

    f32 = mybir.dt.float32
    i32 = mybir.dt.int32
    bf16 = mybir.dt.bfloat16
    mult = mybir.AluOpType.mult

    nc = bacc.Bacc(
        "TRN2",
        target_bir_lowering=False,
        debug=False,
        enable_asserts=False,
        num_devices=NCORES,
    )

    bx_d = nc.dram_tensor("bx", [ROWS, H], bf16, kind="ExternalInput").ap()
    idx_d = nc.dram_tensor("idx", [C, NCHT], i32, kind="ExternalInput").ap()
    # fused gather table: row v = [ae[v] | (ae @ w)[v]] in bf16
    eaw_d = nc.dram_tensor("eaw", [V, 2 * D], bf16, kind="ExternalInput").ap()
    ct_d = nc.dram_tensor("consts", [C, 2 * NCH], f32, kind="ExternalInput").ap()
    b01_d = nc.dram_tensor("base01", [C, C], f32, kind="ExternalInput").ap()
    out_d = nc.dram_tensor("out", [ROWS, H], bf16, kind="ExternalOutput").ap()

    with tile.TileContext(nc) as tc:
        with (
            tc.tile_pool(name="const", bufs=1) as cpool,
            tc.tile_pool(name="bxp", bufs=6) as bxpool,
            tc.tile_pool(name="outp", bufs=4) as outpool,
            tc.tile_pool(name="sm", bufs=4) as smpool,
            tc.tile_pool(name="eap", bufs=6) as eapool,
            tc.tile_pool(name="mp", bufs=2) as mpool,
            tc.tile_pool(name="ps_et", bufs=1, space="PSUM") as ps_et,
            tc.tile_pool(name="ps_at", bufs=1, space="PSUM") as ps_at,
            tc.tile_pool(name="ps_s", bufs=2, space="PSUM") as ps_s,
            tc.tile_pool(name="ps_out", bufs=1, space="PSUM") as ps_out,
            tc.tile_pool(name="ps_m", bufs=1, space="PSUM") as ps_m,
        ):
            ident16 = cpool.tile([C, C], bf16)
            make_identity(nc, ident16[:])
            # idx first: every gather waits on it
            idx_s = cpool.tile([C, NCHT], i32)
            nc.sync.dma_start(out=idx_s[:], in_=idx_d[:, :])
            consts_s = cpool.tile([C, 2 * NCH], f32)
            nc.sync.dma_start(out=consts_s[:], in_=ct_d[:, :])
            b01_s = cpool.tile([C, C], f32)
            nc.sync.dma_start(out=b01_s[:], in_=b01_d[:, :])

            ea_tiles = {}

            def issue_gather(G):
                # row gather: EA[i, :] = [ae[x_i] | aw[x_i]]  [C, 2D]
                EA = eapool.tile([C, 2 * D], bf16, name="EA", tag="EA")
                nc.gpsimd.indirect_dma_start(
                    out=EA[:],
                    out_offset=None,
                    in_=eaw_d[:, :],
                    in_offset=bass.IndirectOffsetOnAxis(
                        ap=idx_s[:, G:G + 1], axis=0
                    ),
                )
                ea_tiles[G] = EA

            issue_gather(0)
            issue_gather(1)

            # software pipeline: chunk G+1's transposes/S interleave into
            # chunk G's big streams
            small = {}

            def emit_small_a(G):
                # E^T, A^T via PE transpose; PSUM->SBUF copies split
                # scalar (E^T) / vector (A^T); Ap on gpsimd from A rows
                c = G % NCH
                EA = ea_tiles[G]
                et_p = ps_et.tile([D, C], bf16, name="et_p", tag="et_p")
                nc.tensor.transpose(out=et_p[:], in_=EA[:, 0:D], identity=ident16[:])
                at_p = ps_at.tile([D, C], bf16, name="at_p", tag="at_p")
                nc.tensor.transpose(
                    out=at_p[:], in_=EA[:, D:2 * D], identity=ident16[:]
                )
                Et = smpool.tile([D, C], bf16, name="Et", tag="Et")
                nc.scalar.copy(out=Et[:], in_=et_p[:])
                At = smpool.tile([D, C], bf16, name="At", tag="At")
                nc.vector.tensor_copy(At[:], at_p[:])
                Ap = smpool.tile([C, D], bf16, name="Ap", tag="Ap")
                nc.gpsimd.tensor_scalar_mul(
                    out=Ap[:], in0=EA[:, D:2 * D], scalar1=consts_s[:, c:c + 1]
                )
                small[G] = {"Et": Et, "At": At, "Ap": Ap}

            def emit_small_b(G):
                # S = A @ E^T  [C, C];  St = S * (gi+1) * mask(i<j)
                c = G % NCH
                s_p = ps_s.tile([C, C], f32, name="s_p", tag="s_p")
                nc.tensor.matmul(
                    out=s_p[:], lhsT=small[G]["At"][:], rhs=small[G]["Et"][:],
                    start=True, stop=True,
                )
                St = smpool.tile([C, C], bf16, name="St", tag="St")
                nc.vector.scalar_tensor_tensor(
                    out=St[:],
                    in0=s_p[:],
                    scalar=consts_s[:, c:c + 1],
                    in1=b01_s[:],
                    op0=mult,
                    op1=mult,
                )
                small[G]["St"] = St

            emit_small_a(0)
            emit_small_b(0)

            M_p = None
            BX2 = OUT2 = None
            M_tiles = {}
            for G in range(NCHT):
                b, c = divmod(G, NCH)
                cc = G % 2
                if c == 0:
                    M_p = ps_m.tile([D, H], f32, name=f"M_p_b{b}", tag="M_p")
                if cc == 0:
                    # one DMA loads two chunks' bx: [256,H] -> [128,2H]
                    BX2 = bxpool.tile([C, 2 * H], bf16, name="BX2", tag="BX2")
                    nc.sync.dma_start(
                        out=BX2[:].rearrange("p (two h) -> p two h", two=2),
                        in_=bx_d[G * C:(G + 2) * C, :].rearrange(
                            "(two p) h -> p two h", two=2
                        ),
                    )
                BX = BX2[:, :H] if cc == 0 else BX2[:, H:]
                if G + 2 < NCHT:
                    issue_gather(G + 2)
                Et = small[G]["Et"]
                St = small[G]["St"]
                Ap = small[G]["Ap"]
                M_s = M_tiles.get(G)

                if G + 1 < NCHT:
                    emit_small_a(G + 1)

                # corr = St^T @ BX (+ E @ M)  [C, H]
                out_p = ps_out.tile([C, H], f32, name="out_p", tag="out_p")
                for lo, hi in ((0, 512), (512, H)):
                    nc.tensor.matmul(
                        out=out_p[:, lo:hi],
                        lhsT=St[:],
                        rhs=BX[:, lo:hi],
                        start=True,
                        stop=(c == 0),
                    )

                if G + 1 < NCHT:
                    emit_small_b(G + 1)

                if c > 0:
                    for lo, hi in ((0, 512), (512, H)):
                        nc.tensor.matmul(
                            out=out_p[:, lo:hi],
                            lhsT=Et[:],
                            rhs=M_s[:, lo:hi],
                            start=False,
                            stop=True,
                        )

                # M += Ap^T @ BX  [D, H]  (skip the never-read last update).
                # skip_group_check: the sim's group guard can't express this
                # read-between-accumulations pattern; Tile's HW sync is
                # unaffected.
                if c < NCH - 1:
                    for lo, hi in ((0, 512), (512, H)):
                        nc.tensor.matmul(
                            out=M_p[:, lo:hi],
                            lhsT=Ap[:],
                            rhs=BX[:, lo:hi],
                            start=(c == 0),
                            stop=True,
                            skip_group_check=True,
                        )

                # out = corr * (1/(gj+1))  (the +bx happens on host);
                # halves split scalar/vector so neither engine saturates
                if cc == 0:
                    OUT2 = outpool.tile([C, 2 * H], bf16, name="OUT2", tag="OUT2")
                out_s = OUT2[:, :H] if cc == 0 else OUT2[:, H:]
                nc.scalar.mul(
                    out=out_s[:, 0:384],
                    in_=out_p[:, 0:384],
                    mul=consts_s[:, NCH + c:NCH + c + 1],
                )
                nc.vector.tensor_scalar_mul(
                    out=out_s[:, 384:H],
                    in0=out_p[:, 384:H],
                    scalar1=consts_s[:, NCH + c:NCH + c + 1],
                )
                if cc == 1:
                    nc.sync.dma_start(
                        out=out_d[(G - 1) * C:(G + 1) * C, :].rearrange(
                            "(two p) h -> p two h", two=2
                        ),
                        in_=OUT2[:].rearrange("p (two h) -> p two h", two=2),
                    )

                # state snapshot for chunk G+1's E@M term (split halves
                # across scalar/vector; reads M_p after this chunk's Mup)
                if G + 1 < NCHT and (G + 1) % NCH > 0:
                    M_s2 = mpool.tile([D, H], bf16, name="M_s", tag="M_s")
                    nc.scalar.copy(out=M_s2[:, 0:384], in_=M_p[:, 0:384])
                    nc.vector.tensor_copy(M_s2[:, 384:H], M_p[:, 384:H])
                    M_tiles[G + 1] = M_s2

    # Adjacent PE matmuls sharing a stationary operand reload it redundantly;
    # mark the second of each such pair as pre-loaded (ldweights=True).
    for blk in nc.m.functions[0].blocks:
        last = None
        for inst in blk.instructions:
            if getattr(inst, "engine", None) != mybir.EngineType.PE:
                continue
            if not isinstance(inst, mybir.InstMatmult):
                if isinstance(inst, (mybir.InstLdweights,)):
                    last = None
                continue
            if (
                last is not None
                and not inst.is_transpose
                and not last.is_transpose
                and inst.ins[1].memref == last.ins[1].memref
                and inst.ins[1].offset == last.ins[1].offset
                and inst.ins[1].ap == last.ins[1].ap
            ):
                inst.ldweights = True
            last = inst

    nc.compile()
    _compiled[key] = nc
    return nc


def _in_maps(bert_x, x, ae, w):
    import ml_dtypes

    bf16 = ml_dtypes.bfloat16
    bert_x = np.ascontiguousarray(np.asarray(bert_x, dtype=np.float32).astype(bf16))
    x = np.asarray(x)
    ae = np.asarray(ae, dtype=np.float32)
    w = np.asarray(w, dtype=np.float32)
    eaw = np.ascontiguousarray(
        np.concatenate([ae, ae @ w], axis=1).astype(bf16)
    )
    base01, consts = _np_consts()
    # idx layout: [C, NCHT] int32, column G = chunk G of the core's batches
    maps = []
    for k in range(NCORES):
        xr = (
            x[k * BPC:(k + 1) * BPC]
            .reshape(NCHT, C)
            .T.astype(np.int32)
        )
        maps.append(
            {
                "bx": bert_x[k * BPC:(k + 1) * BPC].reshape(ROWS, H),
                "idx": np.ascontiguousarray(xr),
                "eaw": eaw,
                "consts": consts,
                "base01": base01,
            }
        )
    return maps


def _run(bert_x, x, ae, w, trace=False):
    from concourse import bass_utils

    nc = _build()
    maps = _in_maps(bert_x, x, ae, w)
    res = bass_utils.run_bass_kernel_spmd(
        nc, maps, core_ids=list(range(NCORES)), trace=trace
    )
    corr = np.concatenate(
        [
            res.results[k]["out"].astype(np.float32).reshape(BPC, L, H)
            for k in range(NCORES)
        ],
        axis=0,
    )
    out = np.asarray(bert_x, dtype=np.float32) + corr
    return out, res


def kernel(bert_x, x, ae, w):
    out, _ = _run(bert_x, x, ae, w, trace=False)
    return out


# revision 8
# speedup vs baseline: 1.1714x; 1.1714x over previous
"""Trainium2 Bass kernel for nn_CausalLayer (bilinear causal mixing layer).

Math (per batch b):
    E = ae[x]                                # [L, D] gather
    S[i,j] = E_i @ w @ E_j                   # bilinear pairwise score
    coef[i,j] = (i+1)/(j+1) for i<j else 0
    res[:,j] = bx[:,j] + sum_i coef[i,j]*S[i,j]*bx[:,i]

Chunked linear-attention decomposition (chunk C=128), with A = E @ w
precomputed on the host into the gather table (row v = [ae[v] | (ae@w)[v]]):
    S_c   = A_c @ E_c^T              [C, C]   local score block
    St_c  = S_c * (gi+1) * mask(i<j)
    M_c   = sum_{c'<c} ((gi+1)A_c')^T @ bx_c'  [D, H]  running state (PSUM)
    corr_c = (St_c^T @ bx_c + E_c @ M_c) / (gj+1)
    res   = bx + corr                          (the +bx happens on the HOST)

Per chunk the PE does: 2 transposes (E^T, A^T), the S block, and three
K<=128 x N=768 streams (St@bx, E@M, M-update) -- ~2240 cycles. The PE on
this part runs at a fixed ~1.2 GHz (HAM clock-gate never unthrottles;
verified with a dense warmup burst), so emission is software-pipelined:
chunk G+1's transposes/S interleave into chunk G's big streams so the
s_p -> St(DVE) -> St@bx chain never stalls the PE.

Output is stored bf16 (gate is 2e-2; bf16 adds ~4e-4) and upcast + bias-
added on the host. Scalar/vector/gpsimd work is split so every engine
stays under the PE window: scalar={E^T copy, fixup-lo, M_s-lo},
vector={A^T copy, St, fixup-hi, M_s-hi}, gpsimd={gathers, Ap scale}.

Sharding: batch-parallel, 2 of 16 batches per NeuronCore across 8 cores;
the gather table / w / masks are replicated. No cross-core communication.
"""

import os
import sys

for _p in ("/opt/trn_rl_repo", "/root/.axon_site/_ro/trn_rl_repo"):
    if os.path.isdir(_p) and _p not in sys.path:
        sys.path.insert(0, _p)

import numpy as np

B, L, H = 16, 2048, 768
V, D = 30000, 64
NCORES = 8
BPC = B // NCORES          # batches per core
C = 128                    # chunk (tile) size along sequence
NCH = L // C               # chunks per batch
ROWS = BPC * L             # bx rows per core
NCHT = BPC * NCH           # chunks per core

_compiled = {}


def _np_consts():
    i = np.arange(C, dtype=np.float64)
    consts = np.zeros((C, 2 * NCH), np.float32)
    for c in range(NCH):
        gi = c * C + i
        consts[:, c] = (gi + 1.0).astype(np.float32)
        consts[:, NCH + c] = (1.0 / (gi + 1.0)).astype(np.float32)
    base01 = (i[:, None] < i[None, :]).astype(np.float32)
    return base01, consts


def _build():
    """Build + compile the per-core Bass module (SPMD: same program, 8 cores)."""
    key = "v4"
    if key in _compiled:
        return _compiled[key]

    import concourse.bacc as bacc
    import concourse.bass as bass
    import concourse.mybir as mybir
    import concourse.tile as tile
    from concourse.masks import make_identity

    f32 = mybir.dt.float32
    i32 = mybir.dt.int32
    bf16 = mybir.dt.bfloat16
    mult = mybir.AluOpType.mult

    nc = bacc.Bacc(
        "TRN2",
        target_bir_lowering=False,
        debug=False,
        enable_asserts=False,
        num_devices=NCORES,
    )

    bx_d = nc.dram_tensor("bx", [ROWS, H], bf16, kind="ExternalInput").ap()
    idx_d = nc.dram_tensor("idx", [C, NCHT], i32, kind="ExternalInput").ap()
    # fused gather table: row v = [ae[v] | (ae @ w)[v]] in bf16
    eaw_d = nc.dram_tensor("eaw", [V, 2 * D], bf16, kind="ExternalInput").ap()
    ct_d = nc.dram_tensor("consts", [C, 2 * NCH], f32, kind="ExternalInput").ap()
    b01_d = nc.dram_tensor("base01", [C, C], f32, kind="ExternalInput").ap()
    out_d = nc.dram_tensor("out", [ROWS, H], bf16, kind="ExternalOutput").ap()

    with tile.TileContext(nc) as tc:
        with (
            tc.tile_pool(name="const", bufs=1) as cpool,
            tc.tile_pool(name="bxp", bufs=6) as bxpool,
            tc.tile_pool(name="outp", bufs=4) as outpool,
            tc.tile_pool(name="sm", bufs=6) as smpool,
            tc.tile_pool(name="eap", bufs=6) as eapool,
            tc.tile_pool(name="mp", bufs=2) as mpool,
            tc.tile_pool(name="ps_et", bufs=1, space="PSUM") as ps_et,
            tc.tile_pool(name="ps_at", bufs=1, space="PSUM") as ps_at,
            tc.tile_pool(name="ps_s", bufs=2, space="PSUM") as ps_s,
            tc.tile_pool(name="ps_out", bufs=1, space="PSUM") as ps_out,
            tc.tile_pool(name="ps_m", bufs=1, space="PSUM") as ps_m,
        ):
            # idx first: every gather waits on it
            idx_s = cpool.tile([C, NCHT], i32)
            nc.sync.dma_start(out=idx_s[:], in_=idx_d[:, :])

            ea_tiles = {}

            def issue_gather(G):
                # row gather: EA[i, :] = [ae[x_i] | aw[x_i]]  [C, 2D]
                EA = eapool.tile([C, 2 * D], bf16, name="EA", tag="EA")
                nc.gpsimd.indirect_dma_start(
                    out=EA[:],
                    out_offset=None,
                    in_=eaw_d[:, :],
                    in_offset=bass.IndirectOffsetOnAxis(
                        ap=idx_s[:, G:G + 1], axis=0
                    ),
                )
                ea_tiles[G] = EA

            for g0 in range(4):
                issue_gather(g0)

            ident16 = cpool.tile([C, C], bf16)
            make_identity(nc, ident16[:])
            consts_s = cpool.tile([C, 2 * NCH], f32)
            nc.sync.dma_start(out=consts_s[:], in_=ct_d[:, :])
            b01_s = cpool.tile([C, C], f32)
            nc.sync.dma_start(out=b01_s[:], in_=b01_d[:, :])

            # software pipeline: chunk G+2's transposes and S-block
            # interleave into chunk G's big streams, so every operand a
            # window consumes was produced >= 1 full window earlier.
            small = {}

            def emit_smalls(G):
                # E^T, A^T via PE transpose; PSUM->SBUF copies split
                # scalar (E^T) / vector (A^T); Ap on vector from A rows
                c = G % NCH
                EA = ea_tiles[G]
                Ap = smpool.tile([C, D], bf16, name="Ap", tag="Ap")
                nc.vector.tensor_scalar_mul(
                    out=Ap[:], in0=EA[:, D:2 * D], scalar1=consts_s[:, c:c + 1]
                )
                et_p = ps_et.tile([D, C], bf16, name="et_p", tag="et_p")
                nc.tensor.transpose(out=et_p[:], in_=EA[:, 0:D], identity=ident16[:])
                at_p = ps_at.tile([D, C], bf16, name="at_p", tag="at_p")
                nc.tensor.transpose(
                    out=at_p[:], in_=EA[:, D:2 * D], identity=ident16[:]
                )
                Et = smpool.tile([D, C], bf16, name="Et", tag="Et")
                nc.scalar.copy(out=Et[:], in_=et_p[:])
                At = smpool.tile([D, C], bf16, name="At", tag="At")
                nc.vector.tensor_copy(At[:], at_p[:])
                small[G] = {"Et": Et, "At": At, "Ap": Ap}

            def emit_sblock(G):
                # S = A @ E^T  [C, C];  St = S * (gi+1) * mask(i<j)
                c = G % NCH
                s_p = ps_s.tile([C, C], f32, name="s_p", tag="s_p")
                nc.tensor.matmul(
                    out=s_p[:], lhsT=small[G]["At"][:], rhs=small[G]["Et"][:],
                    start=True, stop=True,
                )
                St = smpool.tile([C, C], bf16, name="St", tag="St")
                nc.vector.scalar_tensor_tensor(
                    out=St[:],
                    in0=s_p[:],
                    scalar=consts_s[:, c:c + 1],
                    in1=b01_s[:],
                    op0=mult,
                    op1=mult,
                )
                small[G]["St"] = St

            emit_smalls(0)
            emit_sblock(0)
            emit_smalls(1)
            emit_sblock(1)

            M_p = None
            BX2 = OUT2 = None
            M_tiles = {}
            for G in range(NCHT):
                b, c = divmod(G, NCH)
                cc = G % 2
                if c == 0:
                    M_p = ps_m.tile([D, H], f32, name=f"M_p_b{b}", tag="M_p")
                if cc == 0:
                    # one DMA loads two chunks' bx: [256,H] -> [128,2H]
                    BX2 = bxpool.tile([C, 2 * H], bf16, name="BX2", tag="BX2")
                    nc.sync.dma_start(
                        out=BX2[:].rearrange("p (two h) -> p two h", two=2),
                        in_=bx_d[G * C:(G + 2) * C, :].rearrange(
                            "(two p) h -> p two h", two=2
                        ),
                    )
                BX = BX2[:, :H] if cc == 0 else BX2[:, H:]
                if G + 4 < NCHT:
                    issue_gather(G + 4)
                Et = small[G]["Et"]
                St = small[G]["St"]
                Ap = small[G]["Ap"]
                M_s = M_tiles.get(G)

                # PE window order: transposes(G+2) | EtM(G) | S(G+2) |
                # Mup(G) | StBX(G) last -- out_p(G) finishes at the window
                # boundary, giving the fixup a full window to drain before
                # StBX(G+1) reuses the single ps_out slot.
                if G + 2 < NCHT:
                    emit_smalls(G + 2)

                out_p = ps_out.tile([C, H], f32, name="out_p", tag="out_p")
                if c > 0:
                    for lo, hi in ((0, 512), (512, H)):
                        nc.tensor.matmul(
                            out=out_p[:, lo:hi],
                            lhsT=Et[:],
                            rhs=M_s[:, lo:hi],
                            start=True,
                            stop=False,
                        )

                if G + 2 < NCHT:
                    emit_sblock(G + 2)

                # M += Ap^T @ BX  [D, H]  (skip the never-read last update).
                # skip_group_check: the sim's group guard can't express this
                # read-between-accumulations pattern; Tile's HW sync is
                # unaffected.
                if c < NCH - 1:
                    for lo, hi in ((0, 512), (512, H)):
                        nc.tensor.matmul(
                            out=M_p[:, lo:hi],
                            lhsT=Ap[:],
                            rhs=BX[:, lo:hi],
                            start=(c == 0),
                            stop=True,
                            skip_group_check=True,
                        )

                # state snapshot for chunk G+1's E@M term (split halves
                # across scalar/vector; reads M_p right after this Mup)
                if G + 1 < NCHT and (G + 1) % NCH > 0:
                    M_s2 = mpool.tile([D, H], bf16, name="M_s", tag="M_s")
                    nc.scalar.copy(out=M_s2[:, 0:384], in_=M_p[:, 0:384])
                    nc.vector.tensor_copy(M_s2[:, 384:H], M_p[:, 384:H])
                    M_tiles[G + 1] = M_s2

                for lo, hi in ((0, 512), (512, H)):
                    nc.tensor.matmul(
                        out=out_p[:, lo:hi],
                        lhsT=St[:],
                        rhs=BX[:, lo:hi],
                        start=(c == 0),
                        stop=True,
                    )

                # out = corr * (1/(gj+1))  (the +bx happens on host);
                # halves split scalar/vector so neither engine saturates
                if cc == 0:
                    OUT2 = outpool.tile([C, 2 * H], bf16, name="OUT2", tag="OUT2")
                out_s = OUT2[:, :H] if cc == 0 else OUT2[:, H:]
                nc.scalar.mul(
                    out=out_s[:, 0:384],
                    in_=out_p[:, 0:384],
                    mul=consts_s[:, NCH + c:NCH + c + 1],
                )
                nc.vector.tensor_scalar_mul(
                    out=out_s[:, 384:H],
                    in0=out_p[:, 384:H],
                    scalar1=consts_s[:, NCH + c:NCH + c + 1],
                )
                if cc == 1:
                    nc.sync.dma_start(
                        out=out_d[(G - 1) * C:(G + 1) * C, :].rearrange(
                            "(two p) h -> p two h", two=2
                        ),
                        in_=OUT2[:].rearrange("p (two h) -> p two h", two=2),
                    )

    # Adjacent PE matmuls sharing a stationary operand reload it redundantly;
    # mark the second of each such pair as pre-loaded (ldweights=True).
    for blk in nc.m.functions[0].blocks:
        last = None
        for inst in blk.instructions:
            if getattr(inst, "engine", None) != mybir.EngineType.PE:
                continue
            if not isinstance(inst, mybir.InstMatmult):
                if isinstance(inst, (mybir.InstLdweights,)):
                    last = None
                continue
            if (
                last is not None
                and not inst.is_transpose
                and not last.is_transpose
                and inst.ins[1].memref == last.ins[1].memref
                and inst.ins[1].offset == last.ins[1].offset
                and inst.ins[1].ap == last.ins[1].ap
            ):
                inst.ldweights = True
            last = inst

    nc.compile()
    _compiled[key] = nc
    return nc


def _in_maps(bert_x, x, ae, w):
    import ml_dtypes

    bf16 = ml_dtypes.bfloat16
    bert_x = np.ascontiguousarray(np.asarray(bert_x, dtype=np.float32).astype(bf16))
    x = np.asarray(x)
    ae = np.asarray(ae, dtype=np.float32)
    w = np.asarray(w, dtype=np.float32)
    eaw = np.ascontiguousarray(
        np.concatenate([ae, ae @ w], axis=1).astype(bf16)
    )
    base01, consts = _np_consts()
    # idx layout: [C, NCHT] int32, column G = chunk G of the core's batches
    maps = []
    for k in range(NCORES):
        xr = (
            x[k * BPC:(k + 1) * BPC]
            .reshape(NCHT, C)
            .T.astype(np.int32)
        )
        maps.append(
            {
                "bx": bert_x[k * BPC:(k + 1) * BPC].reshape(ROWS, H),
                "idx": np.ascontiguousarray(xr),
                "eaw": eaw,
                "consts": consts,
                "base01": base01,
            }
        )
    return maps


def _run(bert_x, x, ae, w, trace=False):
    from concourse import bass_utils

    nc = _build()
    maps = _in_maps(bert_x, x, ae, w)
    res = bass_utils.run_bass_kernel_spmd(
        nc, maps, core_ids=list(range(NCORES)), trace=trace
    )
    corr = np.concatenate(
        [
            res.results[k]["out"].astype(np.float32).reshape(BPC, L, H)
            for k in range(NCORES)
        ],
        axis=0,
    )
    out = np.asarray(bert_x, dtype=np.float32) + corr
    return out, res


def kernel(bert_x, x, ae, w):
    out, _ = _run(bert_x, x, ae, w, trace=False)
    return out
